# revision 18
# baseline (speedup 1.0000x reference)
"""Multi-head attention (B=8, N=1024, C=768, H=12) on 8 TRN2 NeuronCores.

Sharding: pure data parallelism over the batch — core b computes batch
element b end-to-end (weights replicated); no collectives.

v9 (from v3's ~221us graded / 262us first local measurement, now ~209us
total with Tensor-active ~160us; rel err 0.0097 vs the 2e-2 gate):
  - ALL matmul operands bf16 (qkT, vhat, pt, attn_outT, wprojT). f32r
    matmuls issued ~80ns/instr slower than bf16 (LDWEIGHTS 218 vs 116ns,
    fp32_mode=HIGH streams slower); ~500 of them cost ~35us of Tensor
    queue. bf16 q/k/pt adds ~0.4% rel err — fine against the gate.
  - Softmax normalization commutes with nothing (per-head denominators),
    so it stays on av: denominator via the vhat ones-column, reciprocal
    on partition 0, GpSimd partition_broadcast, DVE mult reading av
    straight from PSUM (the old sv staging copy is gone).
  - No bias matmuls: b_tile is partition-broadcast once at startup and
    added during the proj eviction (DVE tensor_tensor, PSUM + SBUF in,
    bf16 out).
  - Scalar engine does ONLY exp (96 x ~1.1us ACTIVATE, bf16 out) plus
    early input DMA triggers. All PSUM evictions are DVE (GpSimd cannot
    read PSUM). Out-DMAs ride the sync ring.
  - Software-pipelined attention inner loop: score(mc) is emitted TWO
    steps ahead of av(mc) (pt_pool bufs=3 holds exactly the 3 live pt
    tiles). The PE queue is in-order, so an av waiting on exp blocked
    the next score and collapsed the exp stream to (exp+stall) per mc;
    with lag-2 the exps run back-to-back through nh1 and across block
    starts.
  - Proj thunks order kc4 AND kc5 last: those columns of attn_outT come
    from the two most recently normalized pairs, and the DVE norm chain
    (rd -> recip -> pbs -> mult, ~3us) lands only a few us into the
    block that pops them; consuming them early stalled every nh1 block
    start.
  - Proj queue covers mc0-4 (p5-nh0 budget 11 = alloc + kc0-3 matmuls
    only, nh1 blocks budget 9 each); mc4's regular matmuls absorb the
    nh1-p5 exp slack, the rest of mc4 + mc5-7 drain in the tail.
  - DMA: 3 rings. Early window: x and qk weight columns parity-
    interleaved across sync+scalar in phase-A consumption order; then
    v columns on scalar (needed by the p0-block v-projection ~26us in),
    k-tail columns on sync (~55us), wproj alone on gpsimd so the early
    window is not a 3-way HBM fight.
  - Carried from v3: nh-outer loop, lead-1 qk fills inside the nh0
    blocks, vhat ones-column denominator, host-side transposes + bf16
    x/wqkv, bf16 output (host casts back to fp32).
Measurement note: the device clock throttles intermittently (same NEFF
measured 212->250us across processes); test.py profiles 3x and reports
the min.
"""

from contextlib import ExitStack

import numpy as np
import ml_dtypes

import concourse.bass as bass
import concourse.mybir as mybir
import concourse.tile as tile
from concourse import bacc
from concourse.bass_utils import run_bass_kernel_spmd

F32 = mybir.dt.float32
F32R = mybir.dt.float32r
BF16 = mybir.dt.bfloat16

B = 8
N, C, H, D = 1024, 768, 12, 64
F3 = 3 * C
FQK = 2 * C
SCALE = D ** -0.5
NCH = C // 128   # 6 chunks of the contraction dim
NMC = N // 128   # 8 chunks of the sequence dim
NPAIR = H // 2   # 6 head pairs


def _build(nc):
    xT = nc.declare_dram_parameter("xT", [C, N], BF16, isOutput=False)
    wqkvT = nc.declare_dram_parameter("wqkvT", [C, F3], BF16, isOutput=False)
    wprojT = nc.declare_dram_parameter("wprojT", [C, C], BF16, isOutput=False)
    b_proj = nc.declare_dram_parameter("b_proj", [C], F32R, isOutput=False)
    # output in bf16: halves the 3 MB out-DMA (the final transfer sits on
    # the exit-critical path); the host casts back to fp32.
    out = nc.declare_dram_parameter("out", [N, C], BF16, isOutput=True)

    with tile.TileContext(nc) as tc, ExitStack() as ctx:
        const_pool = ctx.enter_context(tc.tile_pool(name="const", bufs=1))
        xw_pool = ctx.enter_context(tc.tile_pool(name="xw", bufs=1))
        qkT_pool = ctx.enter_context(tc.tile_pool(name="qkT", bufs=6))
        vhat_pool = ctx.enter_context(tc.tile_pool(name="vhat", bufs=1))
        aoT_pool = ctx.enter_context(tc.tile_pool(name="aoT", bufs=1))
        pt_pool = ctx.enter_context(tc.tile_pool(name="pt", bufs=3))
        nrm_pool = ctx.enter_context(tc.tile_pool(name="nrm", bufs=1))
        osb_pool = ctx.enter_context(tc.tile_pool(name="osb", bufs=2))

        # ---- input DMA on three rings ----
        xs_all = xw_pool.tile([128, NCH * N], BF16, tag="xs")
        ws_all = xw_pool.tile([128, NCH * F3], BF16, tag="ws")
        xs = [xs_all[:, k * N:(k + 1) * N] for k in range(NCH)]
        ws = [ws_all[:, k * F3:(k + 1) * F3] for k in range(NCH)]
        # ring B (Activation): q/k weight columns — phase A's critical data.
        # ring A (SP): x chunks (earliest consumer), later the out DMAs.
        # ring C (GpSimd): k-tail cols + v cols (needed ~20us in), then w_proj
        # (needed only by the nh=1 pass) and bias.
        # kc=0 split by columns so phase A's first matmul (fc=0, ns=0:
        # needs ws[0][:,0:128] + xs[0][:,0:512]) fires as early as possible.
        for kc in range(NCH):
            if kc == 0:
                nc.scalar.dma_start(ws[0][:, 0:128], wqkvT[0:128, 0:128])
                nc.sync.dma_start(xs[0][:, 0:512], xT[0:128, 0:512])
                nc.scalar.dma_start(ws[0][:, 128:1024], wqkvT[0:128, 128:1024])
                nc.sync.dma_start(xs[0][:, 512:1024], xT[0:128, 512:1024])
            elif kc % 2 == 1:
                nc.scalar.dma_start(xs[kc], xT[kc * 128:(kc + 1) * 128, :])
                nc.sync.dma_start(ws[kc][:, 0:1024],
                                  wqkvT[kc * 128:(kc + 1) * 128, 0:1024])
            else:
                nc.scalar.dma_start(ws[kc][:, 0:1024],
                                    wqkvT[kc * 128:(kc + 1) * 128, 0:1024])
                nc.sync.dma_start(xs[kc], xT[kc * 128:(kc + 1) * 128, :])
        b_row = const_pool.tile([1, C], F32R, tag="b_row")
        nc.scalar.dma_start(b_row[:], b_proj.rearrange("(a o) -> a o", a=1))
        # v columns on the gpsimd queue (first consumer is the
        # v-projection inside the p0 block ~26us in; gpsimd is idle and
        # CAN trigger DMA, it just can't touch PSUM); k-tail cols (fc8-11,
        # first consumed by the p2 fills ~55us in) behind x on the sync
        # ring. wproj triggers are emitted AFTER phase A: ~600ns apiece,
        # 12 early triggers on the scalar queue delayed the phase-A qkT
        # evictions (and the first score) by ~2.5us.
        for kc in range(NCH):
            nc.gpsimd.dma_start(ws[kc][:, FQK:F3],
                                wqkvT[kc * 128:(kc + 1) * 128, FQK:F3])
        for kc in range(NCH):
            nc.sync.dma_start(ws[kc][:, 1024:FQK],
                              wqkvT[kc * 128:(kc + 1) * 128, 1024:FQK])

        wp_all = xw_pool.tile([128, NCH * C], BF16, tag="wp")
        wps = [wp_all[:, k * C:(k + 1) * C] for k in range(NCH)]

        # ---- constants + Exp act-table warm ----
        ones_col_f = const_pool.tile([128, H], BF16, tag="ocf")
        nc.vector.memset(ones_col_f[:], 1.0)
        warm_in = const_pool.tile([1, 8], F32, tag="warmi")
        nc.vector.memset(warm_in[:], 1.0)
        warm = const_pool.tile([1, 8], F32, tag="warm")
        nc.scalar.activation(
            warm[:], warm_in[:], mybir.ActivationFunctionType.Exp,
            bias=0.0, scale=1.0,
        )
        # bias broadcast to all 128 partitions once; added during proj
        # eviction (no PE bias matmuls)
        b_tile = const_pool.tile([128, C], F32R, tag="b_tile")
        nc.gpsimd.partition_broadcast(b_tile[:], b_row[:], channels=128)

        qkT = [None] * 12

        # ---- phase A: qk chunks for pairs 0+1 accumulate during DMA ----
        with tc.tile_pool(name="psA", bufs=4, space="PSUM") as psA:
            pq = {}
            for fc in (0, 6, 1, 7):
                pq[fc] = psA.tile([128, N], F32, tag="ps", name=f"pqA{fc}")
            for kc in range(NCH):
                for fc in (0, 6, 1, 7):
                    for ns in range(2):
                        nc.tensor.matmul(
                            pq[fc][:, ns * 512:(ns + 1) * 512],
                            lhsT=ws[kc][:, fc * 128:(fc + 1) * 128],
                            rhs=xs[kc][:, ns * 512:(ns + 1) * 512],
                            start=(kc == 0), stop=(kc == NCH - 1),
                            skip_group_check=True,
                        )
            for fc, eng in ((0, "act"), (6, "dve"), (1, "act"), (7, "dve")):
                tag = "qkTq" if fc < 6 else "qkTk"
                t = qkT_pool.tile([128, N], BF16, tag=tag, name=f"qkT{fc}")
                if eng == "act":
                    nc.scalar.copy(t[:], pq[fc][:])
                else:
                    nc.vector.tensor_copy(t[:], pq[fc][:])
                qkT[fc] = t
            for kc in range(NCH):
                nc.scalar.dma_start(wps[kc], wprojT[kc * 128:(kc + 1) * 128, :])

        # ---- attention-phase pools (psA closed: 8 banks free) ----
        sc_pool = ctx.enter_context(tc.tile_pool(name="scp", bufs=2, space="PSUM"))
        avp = ctx.enter_context(tc.tile_pool(name="avp", bufs=2, space="PSUM"))
        gen = ctx.enter_context(tc.tile_pool(name="gen", bufs=1, space="PSUM"))

        vhat = [None] * NMC

        def emit_vproj(mc):
            pv = gen.tile([128, N], F32, tag="ps", name=f"pv{mc}")
            for (o0, ow) in ((0, 512), (512, 256)):
                for kc in range(NCH):
                    nc.tensor.matmul(
                        pv[:, o0:o0 + ow],
                        lhsT=xs[kc][:, mc * 128:(mc + 1) * 128],
                        rhs=ws[kc][:, FQK + o0:FQK + o0 + ow],
                        start=(kc == 0), stop=(kc == NCH - 1),
                        skip_group_check=True,
                    )
            vh = vhat_pool.tile([128, H * 128], BF16, tag=f"vhat{mc}",
                                name=f"vh{mc}")
            # per head: cols 0..63 all-ones, cols 64..127 the v values.
            # av row 0 then holds the softmax denominator on PARTITION 0,
            # which reciprocal_approx_fast (a partition-0 custom DVE op)
            # reads straight out of PSUM — no staging copy — while the
            # v-rows land on partitions 64..127, an aligned base for the
            # normalization mult. Rows 1..63 (denominator copies) go
            # unread; the burned PE columns are free (matmul cost is
            # stream rows, not stationary width).
            nc.gpsimd.memset(
                vh.rearrange("p (h e) -> p h e", e=128)[:, :, 0:64], 1.0)
            nc.vector.tensor_copy(
                vh.rearrange("p (h e) -> p h e", e=128)[:, :, 64:128],
                pv[:, 0:C].rearrange("p (h d) -> p h d", d=64),
            )
            vhat[mc] = vh

        # shared queue of deferred proj work popped by p5-nh0 and the nh1
        # blocks (budgeted, so every block keeps the PE streaming and the
        # HAM clock gate never sees an under-filled block)
        proj_queue = []

        def make_qk_thunks(fc, fin_eng="dve"):
            state = {}

            def alloc():
                state["pq"] = gen.tile([128, N], F32, tag="ps", name=f"pq{fc}")

            thunks = [alloc]
            for ns in range(2):
                for kc in range(NCH):
                    def mm(ns=ns, kc=kc):
                        nc.tensor.matmul(
                            state["pq"][:, ns * 512:(ns + 1) * 512],
                            lhsT=ws[kc][:, fc * 128:(fc + 1) * 128],
                            rhs=xs[kc][:, ns * 512:(ns + 1) * 512],
                            start=(kc == 0), stop=(kc == NCH - 1),
                            skip_group_check=True,
                        )
                    thunks.append(mm)

            def fin():
                tag = "qkTq" if fc < 6 else "qkTk"
                t = qkT_pool.tile([128, N], BF16, tag=tag, name=f"qkT{fc}")
                if fin_eng == "act":
                    nc.scalar.copy(t[:], state["pq"][:])
                else:
                    nc.vector.tensor_copy(t[:], state["pq"][:])
                qkT[fc] = t

            thunks.append(fin)
            return thunks

        attn_outT = [
            aoT_pool.tile([128, N], BF16, tag=f"aoT{j}", name=f"aoT{j}")
            for j in range(NCH)
        ]

        def make_proj_thunks(mc, pool, tag, split_fin=False):
            state = {}

            def alloc():
                state["pp"] = pool.tile([128, N], F32, tag=tag, name=f"pp{mc}")

            # kc=4 and kc=5 matmuls LAST: their attn_outT columns come from
            # the two most recent normalizations (pair 4's lands only ~3us
            # into the block that pops these thunks; pair 5's at that
            # block's end), so the in-order PE queue must reach them as
            # late as possible or it stalls on the DVE norm chain
            thunks = [alloc]
            late4, late5 = [], []
            for (o0, ow) in ((0, 512), (512, 256)):
                for kc in range(NCH):
                    def mm(o0=o0, ow=ow, kc=kc):
                        nc.tensor.matmul(
                            state["pp"][:, o0:o0 + ow],
                            lhsT=attn_outT[kc][:, mc * 128:(mc + 1) * 128],
                            rhs=wps[kc][:, o0:o0 + ow],
                            start=(kc == 0), stop=(kc == NCH - 1),
                            skip_group_check=True,
                        )
                    if kc == NCH - 1:
                        late5.append(mm)
                    elif kc == NCH - 2:
                        late4.append(mm)
                    else:
                        thunks.append(mm)
            thunks.extend(late4)
            if split_fin:
                # evict cols 0:512 right after the kc5-o0 matmul (the last
                # writer of those columns) so only the 256-col half sits on
                # the exit-critical path after the final matmul
                def fin_a():
                    ot = osb_pool.tile([128, C], BF16, tag="osb",
                                       name=f"ot{mc}")
                    state["ot"] = ot
                    nc.vector.tensor_tensor(
                        out=ot[:, 0:512], in0=state["pp"][:, 0:512],
                        in1=b_tile[:, 0:512], op=mybir.AluOpType.add,
                    )
                    nc.sync.dma_start(
                        out[mc * 128:(mc + 1) * 128, 0:512], ot[:, 0:512])
                late5.insert(1, fin_a)

                def fin():
                    ot = state["ot"]
                    nc.vector.tensor_tensor(
                        out=ot[:, 512:C], in0=state["pp"][:, 512:C],
                        in1=b_tile[:, 512:C], op=mybir.AluOpType.add,
                    )
                    nc.sync.dma_start(
                        out[mc * 128:(mc + 1) * 128, 512:C], ot[:, 512:C])
            else:
                def fin():
                    ot = osb_pool.tile([128, C], BF16, tag="osb",
                                       name=f"ot{mc}")
                    nc.vector.tensor_tensor(
                        out=ot[:], in0=state["pp"][:, 0:C], in1=b_tile[:],
                        op=mybir.AluOpType.add,
                    )
                    nc.sync.dma_start(out[mc * 128:(mc + 1) * 128, :], ot[:])
            thunks.extend(late5)

            thunks.append(fin)
            return thunks

        # ---- attention: nh-outer; fills = qk chunks (nh0, lead-1) then
        #      proj row-chunks (shared queue: p5-nh0 + nh1 blocks) ----
        for nh in range(2):
            n0 = nh * 512
            for p in range(NPAIR):
                qc = qkT[p]
                kcx = qkT[6 + p]
                fill = []
                pops = [2] * NMC
                budget = None
                if nh == 0:
                    # lead-1: pair p's block computes BOTH of pair p+1's
                    # chunks (p0 carries the v-projection instead). The pop
                    # schedule stretches the 28 thunks across all 8 mc
                    # steps so no step is left without PE filler work.
                    if 1 <= p <= 4:
                        fill = (make_qk_thunks(p + 1, fin_eng="dve")
                                + make_qk_thunks(6 + p + 1, fin_eng="dve"))
                        pops = [4] * NMC
                    elif p == 5:
                        # nh0 half fully done except this pair; proj thunks
                        # are ordered kc5-last so pair 5's own columns are
                        # only consumed after its normalization lands
                        for mcj in range(5):
                            proj_queue.extend(make_proj_thunks(mcj, gen, "ps"))
                        # 11 = alloc + the 10 kc0-4 matmuls: the late kc5
                        # thunks must NOT pop inside this block (pair 5's
                        # nh0 normalization is only emitted at block end)
                        fill = proj_queue
                        budget = 11
                else:
                    fill = proj_queue
                    budget = 9
                av = [
                    avp.tile([128, 512], F32, tag="av", name=f"av{p}_{nh}_{h}")
                    for h in range(2)
                ]
                if p == 0 and nh == 0:
                    emit_vproj(0)
                popped = 0

                def emit_av(mc, pt):
                    for h in range(2):
                        habs = 2 * p + h
                        nc.tensor.matmul(
                            av[h][:],
                            lhsT=vhat[mc][:, habs * 128:habs * 128 + 128],
                            rhs=pt[:, h * 512:(h + 1) * 512],
                            start=(mc == 0), stop=(mc == NMC - 1),
                            skip_group_check=True,
                        )

                # software pipeline: score(mc) is emitted BEFORE av(mc-2).
                # The PE queue is in-order, so with av(mc) directly after
                # score(mc) the av's wait on exp(mc) blocked score(mc+1)
                # and the exp stream went idle — the whole nh1 half ran at
                # (exp + stall) per mc instead of back-to-back exps. Lag 2
                # (pt_pool bufs=3 holds exactly 3 live pt tiles) also
                # hides the exp latency across block-start ramp-in.
                pending = []
                for mc in range(NMC):
                    # pops first: the score below waits on the sc-buffer
                    # recycle (exp mc-2), and the in-order PE queue would
                    # idle on ready filler work queued behind it
                    for _ in range(pops[mc]):
                        if fill and (budget is None or popped < budget):
                            fill.pop(0)()
                            popped += 1
                    sc = sc_pool.tile([128, N], F32, tag="sc",
                                      name=f"sc{p}_{nh}_{mc}")
                    for h in range(2):
                        nc.tensor.matmul(
                            sc[:, h * 512:(h + 1) * 512],
                            lhsT=kcx[h * 64:(h + 1) * 64, mc * 128:(mc + 1) * 128],
                            rhs=qc[h * 64:(h + 1) * 64, n0:n0 + 512],
                            start=True, stop=True,
                            tile_position=(h * 64, 0),
                        )
                    pt = pt_pool.tile([128, N], BF16, tag="pt",
                                      name=f"pt{p}_{nh}_{mc}")
                    nc.scalar.activation(
                        pt[:], sc[:], mybir.ActivationFunctionType.Exp,
                        bias=0.0, scale=float(SCALE),
                    )
                    if p == 0 and nh == 0 and mc + 1 < NMC:
                        # next v-proj chunk fills the exp latency slot
                        emit_vproj(mc + 1)
                    if len(pending) >= 2:
                        emit_av(*pending.pop(0))
                    pending.append((mc, pt))
                for args in pending:
                    emit_av(*args)
                if budget is None:
                    while fill:
                        fill.pop(0)()
                # normalization: zero PE instructions. The chain's latency
                # gates the NEXT pair's first av matmul (avp bufs=2 bank
                # recycle), so it reads the denominator straight from av
                # partition 0 (ones column is first in vhat) — custom-DVE
                # reciprocal_approx_fast reads partition 0 on HW regardless
                # of the input AP's base partition, which is exactly right
                for h in range(2):
                    rf = nrm_pool.tile([1, 512], F32, tag=f"rf{h}",
                                       name=f"rf{p}_{nh}_{h}")
                    nc.vector.reciprocal_approx_fast(rf[:], av[h][0:1, :])
                    pbs = nrm_pool.tile([64, 512], F32, tag=f"pbs{h}",
                                        name=f"pbs{p}_{nh}_{h}")
                    nc.gpsimd.partition_broadcast(pbs[:], rf[:], channels=64)
                    nc.vector.tensor_tensor(
                        out=attn_outT[p][h * 64:(h + 1) * 64, n0:n0 + 512],
                        in0=av[h][64:128, :], in1=pbs[:],
                        op=mybir.AluOpType.mult,
                    )

        # ---- tail: rest of mc4 (from the shared queue) + mc 5..7 ----
        while proj_queue:
            proj_queue.pop(0)()
        for mc in range(5, NMC):
            pool, tag = (gen, "ps") if mc % 2 == 0 else (sc_pool, "sc")
            for t in make_proj_thunks(mc, pool, tag,
                                      split_fin=(mc == NMC - 1)):
                t()

    return nc


_NC_CACHE = None


def _make():
    global _NC_CACHE
    if _NC_CACHE is None:
        nc = bacc.Bacc("TRN2", target_bir_lowering=False, debug=False)
        _build(nc)
        nc.finalize()
        _NC_CACHE = nc
    return _NC_CACHE


def kernel(**inputs):
    x = np.asarray(inputs["x"], dtype=np.float32)
    w_qkv = np.asarray(inputs["w_qkv"], dtype=np.float32)
    w_proj = np.asarray(inputs["w_proj"], dtype=np.float32)
    b_proj = np.asarray(inputs["b_proj"], dtype=np.float32)
    assert x.shape == (B, N, C), x.shape

    bf16 = ml_dtypes.bfloat16
    wqkvT = np.ascontiguousarray(w_qkv.T).astype(bf16)
    wprojT = np.ascontiguousarray(w_proj.T).astype(bf16)
    b_proj = np.ascontiguousarray(b_proj)

    nc = _make()
    in_maps = [
        {"xT": np.ascontiguousarray(x[b].T).astype(bf16), "wqkvT": wqkvT,
         "wprojT": wprojT, "b_proj": b_proj}
        for b in range(B)
    ]
    res = run_bass_kernel_spmd(nc, in_maps, core_ids=list(range(B)))
    return np.stack([res.results[b]["out"] for b in range(B)]).astype(np.float32)


# revision 20
# speedup vs baseline: 1.0338x; 1.0338x over previous
"""Multi-head attention (B=8, N=1024, C=768, H=12) on 8 TRN2 NeuronCores.

Sharding: pure data parallelism over the batch — core b computes batch
element b end-to-end (weights replicated); no collectives.

v11, ~196us total / ~156us Tensor-active (v3 baseline: 221us graded,
262us on this device; rel err 0.0097 vs the 2e-2 gate). Key mechanisms:
  - ALL matmul operands bf16: f32r matmuls issued ~80ns/instr slower
    (LDWEIGHTS 218 vs 116ns + slower streaming); ~500 of them cost
    ~35us of Tensor queue.
  - Software-pipelined attention inner loop: score(mc) emitted TWO
    steps ahead of av(mc) (pt_pool bufs=3 = the 3 live pt tiles). The
    PE queue is in-order, so an av waiting on exp blocked the next
    score and collapsed the exp stream to (exp+stall) per mc; with
    lag-2 the exps run back-to-back. Filler pops are emitted BEFORE the
    score (which waits on the sc-psum recycle) for the same reason.
  - vhat is 128 wide per head: cols 0..63 all-ones, 64..127 = v. So av
    row 0 is the softmax denominator ON PARTITION 0 — read by
    reciprocal_approx_fast (partition-0 custom DVE op) straight from
    PSUM — and the v rows land at partition base 64 (aligned) for the
    normalization mult. Kills the old rd/sv staging copies; chain is
    rf -> partition_broadcast -> mult. Chain latency gates the NEXT
    pair's first av matmul (avp bufs=2 bank recycle), so it matters.
  - Proj thunks order kc4 AND kc5 last: those attn_outT columns come
    from the two most recently normalized pairs (~3us DVE chain);
    consuming them early stalled every block start. Shared proj queue
    covers mc0-4 (p5-nh0 budget 11 keeps the kc5 thunks out of the
    block that hasn't normalized pair 5 yet — popping them earlier
    reads uninitialized SBUF); mc5-7 drain in the tail, the last chunk
    with a split eviction so half overlaps the final matmul.
  - Scalar engine runs ONLY exp (96 x ~1.1us, bf16 out) + early DMA
    triggers; 12 late-input triggers (~600ns each) are kept off its
    early queue (v cols on gpsimd, wproj emitted after the phase-A
    evictions) — they delayed the first score by ~2.5us. All PSUM
    evictions are DVE (GpSimd cannot access PSUM). Out-DMA on sync.
  - No bias matmuls: b_tile partition-broadcast once, added during the
    proj eviction.
  - DMA: x + qk-weight cols parity-interleaved across sync+scalar in
    phase-A consumption order; k-tail behind x on sync; v on gpsimd.
  - Carried from v3: nh-outer loop, lead-1 qk fills in the nh0 blocks,
    vhat ones-column denominator trick, host-side transposes + bf16
    x/wqkv, bf16 output (host casts back to fp32).
Measurement: the device clock throttles intermittently (same NEFF
measured 212->250us across processes); test.py profiles 3x, reports min.
"""

from contextlib import ExitStack

import numpy as np
import ml_dtypes

import concourse.bass as bass
import concourse.mybir as mybir
import concourse.tile as tile
from concourse import bacc
from concourse.bass_utils import run_bass_kernel_spmd

F32 = mybir.dt.float32
F32R = mybir.dt.float32r
BF16 = mybir.dt.bfloat16

B = 8
N, C, H, D = 1024, 768, 12, 64
F3 = 3 * C
FQK = 2 * C
SCALE = D ** -0.5
NCH = C // 128   # 6 chunks of the contraction dim
NMC = N // 128   # 8 chunks of the sequence dim
NPAIR = H // 2   # 6 head pairs


def _build(nc):
    xT = nc.declare_dram_parameter("xT", [C, N], BF16, isOutput=False)
    wqkvT = nc.declare_dram_parameter("wqkvT", [C, F3], BF16, isOutput=False)
    wprojT = nc.declare_dram_parameter("wprojT", [C, C], BF16, isOutput=False)
    b_proj = nc.declare_dram_parameter("b_proj", [C], F32R, isOutput=False)
    # output in bf16: halves the 3 MB out-DMA (the final transfer sits on
    # the exit-critical path); the host casts back to fp32.
    out = nc.declare_dram_parameter("out", [N, C], BF16, isOutput=True)

    with tile.TileContext(nc) as tc, ExitStack() as ctx:
        const_pool = ctx.enter_context(tc.tile_pool(name="const", bufs=1))
        xw_pool = ctx.enter_context(tc.tile_pool(name="xw", bufs=1))
        qkT_pool = ctx.enter_context(tc.tile_pool(name="qkT", bufs=6))
        vhat_pool = ctx.enter_context(tc.tile_pool(name="vhat", bufs=1))
        aoT_pool = ctx.enter_context(tc.tile_pool(name="aoT", bufs=1))
        pt_pool = ctx.enter_context(tc.tile_pool(name="pt", bufs=3))
        nrm_pool = ctx.enter_context(tc.tile_pool(name="nrm", bufs=1))
        osb_pool = ctx.enter_context(tc.tile_pool(name="osb", bufs=2))

        # ---- input DMA on three rings ----
        xs_all = xw_pool.tile([128, NCH * N], BF16, tag="xs")
        ws_all = xw_pool.tile([128, NCH * F3], BF16, tag="ws")
        xs = [xs_all[:, k * N:(k + 1) * N] for k in range(NCH)]
        ws = [ws_all[:, k * F3:(k + 1) * F3] for k in range(NCH)]
        # ring B (Activation): q/k weight columns — phase A's critical data.
        # ring A (SP): x chunks (earliest consumer), later the out DMAs.
        # ring C (GpSimd): k-tail cols + v cols (needed ~20us in), then w_proj
        # (needed only by the nh=1 pass) and bias.
        # kc=0 split by columns so phase A's first matmul (fc=0, ns=0:
        # needs ws[0][:,0:128] + xs[0][:,0:512]) fires as early as possible.
        for kc in range(NCH):
            if kc == 0:
                nc.scalar.dma_start(ws[0][:, 0:128], wqkvT[0:128, 0:128])
                nc.sync.dma_start(xs[0][:, 0:512], xT[0:128, 0:512])
                nc.scalar.dma_start(ws[0][:, 128:1024], wqkvT[0:128, 128:1024])
                nc.sync.dma_start(xs[0][:, 512:1024], xT[0:128, 512:1024])
            elif kc % 2 == 1:
                nc.scalar.dma_start(xs[kc], xT[kc * 128:(kc + 1) * 128, :])
                nc.sync.dma_start(ws[kc][:, 0:1024],
                                  wqkvT[kc * 128:(kc + 1) * 128, 0:1024])
            else:
                nc.scalar.dma_start(ws[kc][:, 0:1024],
                                    wqkvT[kc * 128:(kc + 1) * 128, 0:1024])
                nc.sync.dma_start(xs[kc], xT[kc * 128:(kc + 1) * 128, :])
        b_row = const_pool.tile([1, C], F32R, tag="b_row")
        nc.scalar.dma_start(b_row[:], b_proj.rearrange("(a o) -> a o", a=1))
        # ALL late inputs ride the sync ring in first-use order (v for the
        # p0 v-projection ~27us, k-tail for the p2 fills ~55us, wproj for
        # the proj queue ~100us). The tile scheduler HOISTS dependency-
        # free DMA triggers ahead of blocked work on an engine's queue,
        # so any trigger left on the scalar ring lands exactly when the
        # phase-A qkT evictions become ready and delays the first score
        # (~600ns per trigger); sync is otherwise idle all run.
        for kc in range(NCH):
            nc.sync.dma_start(ws[kc][:, FQK:F3],
                              wqkvT[kc * 128:(kc + 1) * 128, FQK:F3])
        for kc in range(NCH):
            nc.sync.dma_start(ws[kc][:, 1024:FQK],
                              wqkvT[kc * 128:(kc + 1) * 128, 1024:FQK])

        wp_all = xw_pool.tile([128, NCH * C], BF16, tag="wp")
        wps = [wp_all[:, k * C:(k + 1) * C] for k in range(NCH)]
        for kc in range(NCH):
            nc.sync.dma_start(wps[kc], wprojT[kc * 128:(kc + 1) * 128, :])

        # ---- constants + Exp act-table warm ----
        ones_col_f = const_pool.tile([128, H], BF16, tag="ocf")
        nc.vector.memset(ones_col_f[:], 1.0)
        warm_in = const_pool.tile([1, 8], F32, tag="warmi")
        nc.vector.memset(warm_in[:], 1.0)
        warm = const_pool.tile([1, 8], F32, tag="warm")
        nc.scalar.activation(
            warm[:], warm_in[:], mybir.ActivationFunctionType.Exp,
            bias=0.0, scale=1.0,
        )
        # bias broadcast to all 128 partitions once; added during proj
        # eviction (no PE bias matmuls)
        b_tile = const_pool.tile([128, C], F32R, tag="b_tile")
        nc.gpsimd.partition_broadcast(b_tile[:], b_row[:], channels=128)

        qkT = [None] * 12

        # ---- phase A: qk chunks for pairs 0+1 accumulate during DMA ----
        with tc.tile_pool(name="psA", bufs=4, space="PSUM") as psA:
            pq = {}
            for fc in (0, 6, 1, 7):
                pq[fc] = psA.tile([128, N], F32, tag="ps", name=f"pqA{fc}")
            for kc in range(NCH):
                for fc in (0, 6, 1, 7):
                    for ns in range(2):
                        nc.tensor.matmul(
                            pq[fc][:, ns * 512:(ns + 1) * 512],
                            lhsT=ws[kc][:, fc * 128:(fc + 1) * 128],
                            rhs=xs[kc][:, ns * 512:(ns + 1) * 512],
                            start=(kc == 0), stop=(kc == NCH - 1),
                            skip_group_check=True,
                        )
            for fc, eng in ((0, "act"), (6, "dve"), (1, "act"), (7, "dve")):
                tag = "qkTq" if fc < 6 else "qkTk"
                t = qkT_pool.tile([128, N], BF16, tag=tag, name=f"qkT{fc}")
                if eng == "act":
                    nc.scalar.copy(t[:], pq[fc][:])
                else:
                    nc.vector.tensor_copy(t[:], pq[fc][:])
                qkT[fc] = t

        # ---- attention-phase pools (psA closed: 8 banks free) ----
        sc_pool = ctx.enter_context(tc.tile_pool(name="scp", bufs=2, space="PSUM"))
        avp = ctx.enter_context(tc.tile_pool(name="avp", bufs=2, space="PSUM"))
        gen = ctx.enter_context(tc.tile_pool(name="gen", bufs=1, space="PSUM"))

        vhat = [None] * NMC

        def emit_vproj(mc):
            pv = gen.tile([128, N], F32, tag="ps", name=f"pv{mc}")
            for (o0, ow) in ((0, 512), (512, 256)):
                for kc in range(NCH):
                    nc.tensor.matmul(
                        pv[:, o0:o0 + ow],
                        lhsT=xs[kc][:, mc * 128:(mc + 1) * 128],
                        rhs=ws[kc][:, FQK + o0:FQK + o0 + ow],
                        start=(kc == 0), stop=(kc == NCH - 1),
                        skip_group_check=True,
                    )
            vh = vhat_pool.tile([128, H * 128], BF16, tag=f"vhat{mc}",
                                name=f"vh{mc}")
            # per head: cols 0..63 all-ones, cols 64..127 the v values.
            # av row 0 then holds the softmax denominator on PARTITION 0,
            # which reciprocal_approx_fast (a partition-0 custom DVE op)
            # reads straight out of PSUM — no staging copy — while the
            # v-rows land on partitions 64..127, an aligned base for the
            # normalization mult. Rows 1..63 (denominator copies) go
            # unread; the burned PE columns are free (matmul cost is
            # stream rows, not stationary width).
            nc.gpsimd.memset(
                vh.rearrange("p (h e) -> p h e", e=128)[:, :, 0:64], 1.0)
            nc.vector.tensor_copy(
                vh.rearrange("p (h e) -> p h e", e=128)[:, :, 64:128],
                pv[:, 0:C].rearrange("p (h d) -> p h d", d=64),
            )
            vhat[mc] = vh

        # shared queue of deferred proj work popped by p5-nh0 and the nh1
        # blocks (budgeted, so every block keeps the PE streaming and the
        # HAM clock gate never sees an under-filled block)
        proj_queue = []

        def make_qk_thunks(fc, fin_eng="dve"):
            state = {}

            def alloc():
                state["pq"] = gen.tile([128, N], F32, tag="ps", name=f"pq{fc}")

            thunks = [alloc]
            for ns in range(2):
                for kc in range(NCH):
                    def mm(ns=ns, kc=kc):
                        nc.tensor.matmul(
                            state["pq"][:, ns * 512:(ns + 1) * 512],
                            lhsT=ws[kc][:, fc * 128:(fc + 1) * 128],
                            rhs=xs[kc][:, ns * 512:(ns + 1) * 512],
                            start=(kc == 0), stop=(kc == NCH - 1),
                            skip_group_check=True,
                        )
                    thunks.append(mm)

            def fin():
                tag = "qkTq" if fc < 6 else "qkTk"
                t = qkT_pool.tile([128, N], BF16, tag=tag, name=f"qkT{fc}")
                if fin_eng == "act":
                    nc.scalar.copy(t[:], state["pq"][:])
                else:
                    nc.vector.tensor_copy(t[:], state["pq"][:])
                qkT[fc] = t

            thunks.append(fin)
            return thunks

        attn_outT = [
            aoT_pool.tile([128, N], BF16, tag=f"aoT{j}", name=f"aoT{j}")
            for j in range(NCH)
        ]

        def make_proj_thunks(mc, pool, tag, split_fin=False):
            state = {}

            def alloc():
                state["pp"] = pool.tile([128, N], F32, tag=tag, name=f"pp{mc}")

            # kc=4 and kc=5 matmuls LAST: their attn_outT columns come from
            # the two most recent normalizations (pair 4's lands only ~3us
            # into the block that pops these thunks; pair 5's at that
            # block's end), so the in-order PE queue must reach them as
            # late as possible or it stalls on the DVE norm chain
            thunks = [alloc]
            late4, late5 = [], []
            for (o0, ow) in ((0, 512), (512, 256)):
                for kc in range(NCH):
                    def mm(o0=o0, ow=ow, kc=kc):
                        nc.tensor.matmul(
                            state["pp"][:, o0:o0 + ow],
                            lhsT=attn_outT[kc][:, mc * 128:(mc + 1) * 128],
                            rhs=wps[kc][:, o0:o0 + ow],
                            start=(kc == 0), stop=(kc == NCH - 1),
                            skip_group_check=True,
                        )
                    if kc == NCH - 1:
                        late5.append(mm)
                    elif kc == NCH - 2:
                        late4.append(mm)
                    else:
                        thunks.append(mm)
            thunks.extend(late4)
            if split_fin:
                # evict cols 0:512 right after the kc5-o0 matmul (the last
                # writer of those columns) so only the 256-col half sits on
                # the exit-critical path after the final matmul
                def fin_a():
                    ot = osb_pool.tile([128, C], BF16, tag="osb",
                                       name=f"ot{mc}")
                    state["ot"] = ot
                    nc.vector.tensor_tensor(
                        out=ot[:, 0:512], in0=state["pp"][:, 0:512],
                        in1=b_tile[:, 0:512], op=mybir.AluOpType.add,
                    )
                    nc.sync.dma_start(
                        out[mc * 128:(mc + 1) * 128, 0:512], ot[:, 0:512])
                late5.insert(1, fin_a)

                def fin():
                    ot = state["ot"]
                    nc.vector.tensor_tensor(
                        out=ot[:, 512:C], in0=state["pp"][:, 512:C],
                        in1=b_tile[:, 512:C], op=mybir.AluOpType.add,
                    )
                    nc.sync.dma_start(
                        out[mc * 128:(mc + 1) * 128, 512:C], ot[:, 512:C])
            else:
                def fin():
                    ot = osb_pool.tile([128, C], BF16, tag="osb",
                                       name=f"ot{mc}")
                    nc.vector.tensor_tensor(
                        out=ot[:], in0=state["pp"][:, 0:C], in1=b_tile[:],
                        op=mybir.AluOpType.add,
                    )
                    nc.sync.dma_start(out[mc * 128:(mc + 1) * 128, :], ot[:])
            thunks.extend(late5)

            thunks.append(fin)
            return thunks

        # ---- attention: nh-outer; fills = qk chunks (nh0, lead-1) then
        #      proj row-chunks (shared queue: p5-nh0 + nh1 blocks) ----
        for nh in range(2):
            n0 = nh * 512
            for p in range(NPAIR):
                qc = qkT[p]
                kcx = qkT[6 + p]
                fill = []
                pops = [2] * NMC
                budget = None
                if nh == 0:
                    # lead-1: pair p's block computes BOTH of pair p+1's
                    # chunks (p0 carries the v-projection instead). The pop
                    # schedule stretches the 28 thunks across all 8 mc
                    # steps so no step is left without PE filler work.
                    if 1 <= p <= 4:
                        fill = (make_qk_thunks(p + 1, fin_eng="dve")
                                + make_qk_thunks(6 + p + 1, fin_eng="dve"))
                        pops = [4] * NMC
                    elif p == 5:
                        # nh0 half fully done except this pair; proj thunks
                        # are ordered kc5-last so pair 5's own columns are
                        # only consumed after its normalization lands
                        for mcj in range(5):
                            proj_queue.extend(make_proj_thunks(mcj, gen, "ps"))
                        # 11 = alloc + the 10 kc0-4 matmuls: the late kc5
                        # thunks must NOT pop inside this block (pair 5's
                        # nh0 normalization is only emitted at block end)
                        fill = proj_queue
                        budget = 11
                else:
                    fill = proj_queue
                    budget = 9
                av = [
                    avp.tile([128, 512], F32, tag="av", name=f"av{p}_{nh}_{h}")
                    for h in range(2)
                ]
                if p == 0 and nh == 0:
                    emit_vproj(0)
                popped = 0

                def emit_av(mc, pt):
                    for h in range(2):
                        habs = 2 * p + h
                        nc.tensor.matmul(
                            av[h][:],
                            lhsT=vhat[mc][:, habs * 128:habs * 128 + 128],
                            rhs=pt[:, h * 512:(h + 1) * 512],
                            start=(mc == 0), stop=(mc == NMC - 1),
                            skip_group_check=True,
                        )

                # software pipeline: score(mc) is emitted BEFORE av(mc-2).
                # The PE queue is in-order, so with av(mc) directly after
                # score(mc) the av's wait on exp(mc) blocked score(mc+1)
                # and the exp stream went idle — the whole nh1 half ran at
                # (exp + stall) per mc instead of back-to-back exps. Lag 2
                # (pt_pool bufs=3 holds exactly 3 live pt tiles) also
                # hides the exp latency across block-start ramp-in.
                pending = []
                for mc in range(NMC):
                    # pops first: the score below waits on the sc-buffer
                    # recycle (exp mc-2), and the in-order PE queue would
                    # idle on ready filler work queued behind it
                    for _ in range(pops[mc]):
                        if fill and (budget is None or popped < budget):
                            fill.pop(0)()
                            popped += 1
                    sc = sc_pool.tile([128, N], F32, tag="sc",
                                      name=f"sc{p}_{nh}_{mc}")
                    for h in range(2):
                        nc.tensor.matmul(
                            sc[:, h * 512:(h + 1) * 512],
                            lhsT=kcx[h * 64:(h + 1) * 64, mc * 128:(mc + 1) * 128],
                            rhs=qc[h * 64:(h + 1) * 64, n0:n0 + 512],
                            start=True, stop=True,
                            tile_position=(h * 64, 0),
                        )
                    pt = pt_pool.tile([128, N], BF16, tag="pt",
                                      name=f"pt{p}_{nh}_{mc}")
                    nc.scalar.activation(
                        pt[:], sc[:], mybir.ActivationFunctionType.Exp,
                        bias=0.0, scale=float(SCALE),
                    )
                    if p == 0 and nh == 0 and mc + 1 < NMC:
                        # next v-proj chunk fills the exp latency slot
                        emit_vproj(mc + 1)
                    if len(pending) >= 2:
                        emit_av(*pending.pop(0))
                    pending.append((mc, pt))
                for args in pending:
                    emit_av(*args)
                if budget is None:
                    while fill:
                        fill.pop(0)()
                # normalization: zero PE instructions. The chain's latency
                # gates the NEXT pair's first av matmul (avp bufs=2 bank
                # recycle), so it reads the denominator straight from av
                # partition 0 (ones column is first in vhat) — custom-DVE
                # reciprocal_approx_fast reads partition 0 on HW regardless
                # of the input AP's base partition, which is exactly right
                for h in range(2):
                    rf = nrm_pool.tile([1, 512], F32, tag=f"rf{h}",
                                       name=f"rf{p}_{nh}_{h}")
                    nc.vector.reciprocal_approx_fast(rf[:], av[h][0:1, :])
                    pbs = nrm_pool.tile([64, 512], F32, tag=f"pbs{h}",
                                        name=f"pbs{p}_{nh}_{h}")
                    nc.gpsimd.partition_broadcast(pbs[:], rf[:], channels=64)
                    nc.vector.tensor_tensor(
                        out=attn_outT[p][h * 64:(h + 1) * 64, n0:n0 + 512],
                        in0=av[h][64:128, :], in1=pbs[:],
                        op=mybir.AluOpType.mult,
                    )

        # ---- tail: rest of mc4 (from the shared queue) + mc 5..7 ----
        while proj_queue:
            proj_queue.pop(0)()
        for mc in range(5, NMC):
            pool, tag = (gen, "ps") if mc % 2 == 0 else (sc_pool, "sc")
            for t in make_proj_thunks(mc, pool, tag,
                                      split_fin=(mc == NMC - 1)):
                t()

    return nc


_NC_CACHE = None


def _make():
    global _NC_CACHE
    if _NC_CACHE is None:
        nc = bacc.Bacc("TRN2", target_bir_lowering=False, debug=False)
        _build(nc)
        nc.finalize()
        _NC_CACHE = nc
    return _NC_CACHE


def kernel(**inputs):
    x = np.asarray(inputs["x"], dtype=np.float32)
    w_qkv = np.asarray(inputs["w_qkv"], dtype=np.float32)
    w_proj = np.asarray(inputs["w_proj"], dtype=np.float32)
    b_proj = np.asarray(inputs["b_proj"], dtype=np.float32)
    assert x.shape == (B, N, C), x.shape

    bf16 = ml_dtypes.bfloat16
    wqkvT = np.ascontiguousarray(w_qkv.T).astype(bf16)
    wprojT = np.ascontiguousarray(w_proj.T).astype(bf16)
    b_proj = np.ascontiguousarray(b_proj)

    nc = _make()
    in_maps = [
        {"xT": np.ascontiguousarray(x[b].T).astype(bf16), "wqkvT": wqkvT,
         "wprojT": wprojT, "b_proj": b_proj}
        for b in range(B)
    ]
    res = run_bass_kernel_spmd(nc, in_maps, core_ids=list(range(B)))
    return np.stack([res.results[b]["out"] for b in range(B)]).astype(np.float32)
